# revision 11
# baseline (speedup 1.0000x reference)
"""Trainium2 Bass kernel for segmented linear (performer-style) attention.

Problem: nn_Attention_43550968382196 (sparse_attention).
  N=32768 tokens in 64 contiguous equal segments of 512, d_qk=128, d_v=256,
  m=256 random features.  Per segment:
     phi_q = (exp(Uq - hq - rowmax(Uq)) + eps) / sqrt(m)
     phi_k = (exp(Uk - hk - segmax(Uk)) + eps) / sqrt(m)
     out   = (phi_q @ (phi_k^T V)) / (phi_q . sum(phi_k) + 1e-8)

Device math (v5; validated 4.8e-3 rel err vs the jax reference in numpy):
  * All matmuls bf16 operands, fp32 PSUM accumulation (fp8 was tested and
    fails the 2e-2 gate: e4m3's 6% per-element error survives averaging).
  * Q side: Qp = exp(Uq - hq - mx) via one Act pass per chunk (bias AP);
    the +eps rides the PSUM->SBUF copy after the PE transpose (Copy with
    float bias / tensor_scalar_add), so no eps rank-1 on the Q side.
  * K side: e^{-hk} is folded into V rows ON THE HOST (V' = e^{-hk} V), so
    phi_k-dev = exp(Uk) needs NO bias: one exp covers 2 chunks.  The
    segment max enters only through the eps correction: segmax' =
    max(exp(Uk)) = e^{segmax} via a gpsimd all-dims reduce, used as the
    rank-1 lhsT scale: KV += segmax' * (eps * colsum_raw(V)), Ksum +=
    segmax' * eps*512.  Per-segment scale e^{segmax} cancels in the ratio.
  * den is a separate 1-column matmul chain; num/den are DMA'd out in raw
    fp32 straight from PSUM and the division (+ the 1e-8*m*segmax' norm
    epsilon) happens on the host.

Sharding: 64 segments split 8-per-core across 8 NeuronCores (data parallel,
no collectives); each core runs this program on its 4096-token shard.
"""

import math
import os
import sys

for _p in ("/opt/trn_rl_repo",):
    if _p not in sys.path and os.path.isdir(_p):
        sys.path.insert(0, _p)

import numpy as np
import ml_dtypes

import concourse.bass as bass
import concourse.bacc as bacc
import concourse.tile as tile
from concourse import mybir
from concourse.bass_utils import run_bass_kernel_spmd

F32 = mybir.dt.float32
BF16 = mybir.dt.bfloat16
AF = mybir.ActivationFunctionType
ALU = mybir.AluOpType
AX = mybir.AxisListType

N_CORES = 8
N = 32768
D = 128          # qk dim
M = 256          # features
DV = 256         # v dim
P = 128          # partitions / tokens per chunk
NSEG = 8         # segments per core
CH = 4           # chunks per segment
MC = 2           # m chunks (256 / 128)
TOK = NSEG * 512

EPS_PHI = 1e-4
EPS_NORM = 1e-8


def build_nc():
    nc = bacc.Bacc("TRN2", target_bir_lowering=False, debug=False)

    QTd = nc.declare_dram_parameter("QT", [D, TOK], BF16, isOutput=False)
    KTd = nc.declare_dram_parameter("KT", [D, TOK], BF16, isOutput=False)
    Vd = nc.declare_dram_parameter("VP", [TOK, DV], BF16, isOutput=False)
    Wd = nc.declare_dram_parameter("omega", [D, M], BF16, isOutput=False)
    HQd = nc.declare_dram_parameter("HQM", [P, NSEG * CH], F32, isOutput=False)
    EHd = nc.declare_dram_parameter("EHK", [P, NSEG * CH], BF16, isOutput=False)
    CVd = nc.declare_dram_parameter("CVS", [1, NSEG * DV], BF16, isOutput=False)
    Id = nc.declare_dram_parameter("identb", [P, P], BF16, isOutput=False)
    Od = nc.declare_dram_parameter("num", [TOK, DV], BF16, isOutput=True)
    Dd = nc.declare_dram_parameter("den", [P, NSEG * CH], F32, isOutput=True)
    Sd = nc.declare_dram_parameter("smax", [1, NSEG], F32, isOutput=True)

    Vv = Vd[:, :].rearrange("(s c p) d -> s p c d", s=NSEG, c=CH, p=P)
    Ov = Od[:, :].rearrange("(s c p) d -> s p c d", s=NSEG, c=CH, p=P)

    with tile.TileContext(nc) as tc:
        with (
            tc.tile_pool(name="const", bufs=1) as const,
            tc.tile_pool(name="sb", bufs=2) as sb,
            tc.tile_pool(name="sm", bufs=3) as sm,
            tc.tile_pool(name="ps", bufs=1, space="PSUM") as ps,
        ):
            omega_t = const.tile([D, M], BF16, name="omega_t")
            nc.sync.dma_start(omega_t[:, :], Wd[:, :])
            ident_t = const.tile([P, P], BF16, name="ident_t")
            nc.sync.dma_start(ident_t[:, :], Id[:, :])
            hqm_t = const.tile([P, NSEG, CH], F32, name="hqm_t")
            nc.sync.dma_start(
                hqm_t[:, :, :],
                HQd[:, :].rearrange("p (s c) -> p s c", s=NSEG))
            ehk_t = const.tile([P, NSEG, CH], BF16, name="ehk_t")
            nc.sync.dma_start(
                ehk_t[:, :, :],
                EHd[:, :].rearrange("p (s c) -> p s c", s=NSEG))
            cvs_t = const.tile([1, NSEG, DV], BF16, name="cvs_t")
            nc.sync.dma_start(
                cvs_t[:, :, :],
                CVd[:, :].rearrange("p (s d) -> p s d", s=NSEG))
            c512_t = const.tile([1, 1], BF16, name="c512_t")
            nc.vector.memset(c512_t[:, :], EPS_PHI * 512.0)
            # outputs accumulated in SBUF, one DMA each at the end
            denAll = const.tile([P, NSEG, CH], F32, name="denAll")
            smaxAll = const.tile([1, NSEG], F32, name="smaxAll")

            # per-segment input loads so segment 0 compute starts right away
            qT_all = const.tile([D, TOK], BF16, name="qT_all")
            kT_all = const.tile([D, TOK], BF16, name="kT_all")
            vp_all = const.tile([P, NSEG, CH, DV], BF16, name="vp_all")
            for s in range(NSEG):
                sl = bass.ts(s, 512)
                nc.sync.dma_start(kT_all[:, sl], KTd[:, sl])
                nc.sync.dma_start(qT_all[:, sl], QTd[:, sl])
                nc.sync.dma_start(vp_all[:, s], Vv[s])

            # per-segment state carried between pipeline stages
            st = [None] * NSEG

            def stage1(s):
                # ---- K side first: its smax chain is the longest ------
                uk0 = ps.tile([P, 2, M], F32, name=f"uk0_{s}", tag="U", bufs=3)
                uk1 = ps.tile([P, 2, M], F32, name=f"uk1_{s}", tag="U", bufs=3)
                for c in range(CH):
                    u = (uk0, uk1)[c // 2]
                    nc.tensor.matmul(u[:, c % 2, :],
                                     kT_all[:, bass.ts(s * CH + c, P)],
                                     omega_t[:, :])
                # K: exp with no bias (one op per psU tile)
                kp = sb.tile([P, CH, M], BF16, name=f"kp{s}", tag="kp", bufs=3)
                nc.scalar.activation(kp[:, 0:2, :], uk0[:, :, :], AF.Exp)
                nc.scalar.activation(kp[:, 2:4, :], uk1[:, :, :], AF.Exp)
                # segmax' = max(exp(Uk)) via gpsimd all-reduce (SBUF in)
                smx = sm.tile([1, 1], F32, name=f"smx{s}", tag="smx")
                nc.gpsimd.tensor_reduce(smx[:, :], kp[:, :, :],
                                        axis=AX.XYZWC, op=ALU.max)
                smrow = sm.tile([1, P], BF16, name=f"smrow{s}", tag="smrow")
                nc.vector.tensor_copy(smrow[:, :],
                                      smx[:, :].broadcast_to([1, P]))
                nc.vector.tensor_copy(smaxAll[0:1, s:s + 1], smx[:, :])

                # ---- Q side: U matmuls, rowmax -> bias -> exp ---------
                uq0 = ps.tile([P, 2, M], F32, name=f"uq0_{s}", tag="U", bufs=3)
                uq1 = ps.tile([P, 2, M], F32, name=f"uq1_{s}", tag="U", bufs=3)
                for c in range(CH):
                    u = (uq0, uq1)[c // 2]
                    nc.tensor.matmul(u[:, c % 2, :],
                                     qT_all[:, bass.ts(s * CH + c, P)],
                                     omega_t[:, :])
                mx4 = sm.tile([P, CH], F32, name=f"mx4_{s}", tag="mx4")
                nc.vector.tensor_reduce(mx4[:, 0:2], uq0[:, :, :],
                                        axis=AX.X, op=ALU.max)
                nc.vector.tensor_reduce(mx4[:, 2:4], uq1[:, :, :],
                                        axis=AX.X, op=ALU.max)
                biasq = sm.tile([P, CH], F32, name=f"biasq_{s}", tag="biasq")
                nc.vector.tensor_tensor(biasq[:, :], hqm_t[:, s], mx4[:, :],
                                        op=ALU.subtract)
                qp = sb.tile([P, CH, M], BF16, name=f"qp{s}", tag="qp", bufs=3)
                for c in range(CH):
                    nc.scalar.activation(qp[:, c, :],
                                         (uq0, uq1)[c // 2][:, c % 2, :],
                                         AF.Exp, bias=biasq[:, c:c + 1])
                st[s] = (qp, kp, smrow)

            def stage2a(s):
                qp, kp, smrow = st[s]
                # ---- QpT = T(qp) + eps  (PE transpose, copy adds eps) -
                psT0 = ps.tile([P, 512], BF16, name=f"psT0_{s}", tag="T",
                               bufs=2)
                psT1 = ps.tile([P, 512], BF16, name=f"psT1_{s}", tag="T",
                               bufs=2)
                for c in range(CH):
                    nc.tensor.transpose(psT0[:, bass.ts(c, P)],
                                        qp[:, c, 0:P], ident_t[:, :])
                    nc.tensor.transpose(psT1[:, bass.ts(c, P)],
                                        qp[:, c, P:M], ident_t[:, :])
                qpT = sb.tile([P, MC, 512], BF16, name=f"qpT{s}", tag="qpT",
                              bufs=2)
                nc.scalar.activation(qpT[:, 0, :], psT0[:, :], AF.Copy,
                                     bias=EPS_PHI)
                nc.vector.tensor_scalar_add(qpT[:, 1, :], psT1[:, :],
                                            EPS_PHI)

                # ---- KV = Kp^T V' (+ rank-1 eps) ; Ksum likewise ------
                psKV = ps.tile([P, MC, DV], F32, name=f"psKV{s}", tag="W",
                               bufs=1)
                psKs = ps.tile([P, MC, 1], F32, name=f"psKs{s}", tag="T",
                               bufs=2)
                for mc in range(MC):
                    nc.tensor.matmul(psKV[:, mc, :], smrow[0:1, :],
                                     cvs_t[0:1, s, :], start=True, stop=False)
                    for c in range(CH):
                        nc.tensor.matmul(psKV[:, mc, :],
                                         kp[:, c, bass.ts(mc, P)],
                                         vp_all[:, s, c, :],
                                         start=False, stop=(c == CH - 1))
                    nc.tensor.matmul(psKs[:, mc, :], smrow[0:1, :],
                                     c512_t[0:1, :], start=True, stop=False)
                    for c in range(CH):
                        nc.tensor.matmul(psKs[:, mc, :],
                                         kp[:, c, bass.ts(mc, P)],
                                         ehk_t[:, s, c:c + 1],
                                         start=False, stop=(c == CH - 1))
                kvb = sb.tile([P, MC, DV + 1], BF16, name=f"kvb{s}",
                              tag="kvb", bufs=2)
                nc.scalar.activation(kvb[:, 0, 0:DV], psKV[:, 0, :], AF.Copy)
                nc.vector.tensor_copy(kvb[:, 1, 0:DV], psKV[:, 1, :])
                nc.vector.tensor_copy(kvb[:, :, DV:DV + 1], psKs[:, :, :])
                st[s] = (qpT, kvb)

            def stage2b(s):
                qpT, kvb = st[s]
                # ---- num / den matmuls + stores -----------------------
                psD = ps.tile([P, CH], F32, name=f"psD{s}", tag="T", bufs=2)
                for half in range(2):
                    psN = ps.tile([P, 2, DV], F32, name=f"psN{s}_{half}",
                                  tag="NN", bufs=2)
                    for i in range(2):
                        c = half * 2 + i
                        for mc in range(MC):
                            nc.tensor.matmul(psN[:, i, :],
                                             qpT[:, mc, bass.ts(c, P)],
                                             kvb[:, mc, 0:DV],
                                             start=(mc == 0), stop=(mc == 1))
                        for mc in range(MC):
                            nc.tensor.matmul(psD[:, c:c + 1],
                                             qpT[:, mc, bass.ts(c, P)],
                                             kvb[:, mc, DV:DV + 1],
                                             start=(mc == 0), stop=(mc == 1))
                    numb = sb.tile([P, 2, DV], BF16, name=f"numb{s}_{half}",
                                   tag="numb", bufs=3)
                    if half == 0:
                        nc.scalar.activation(numb[:, :, :], psN[:, :, :],
                                             AF.Copy)
                    else:
                        nc.vector.tensor_copy(numb[:, :, :], psN[:, :, :])
                    nc.sync.dma_start(Ov[s, :, 2 * half:2 * half + 2, :],
                                      numb[:, :, :])
                nc.vector.tensor_copy(denAll[:, s, :], psD[:, :])

            for s in range(NSEG):
                if s > 0:
                    stage2a(s - 1)
                stage1(s)
                if s > 0:
                    stage2b(s - 1)
            stage2a(NSEG - 1)
            stage2b(NSEG - 1)

            nc.sync.dma_start(Dd[:, :],
                              denAll[:, :, :].rearrange("p s c -> p (s c)"))
            nc.sync.dma_start(Sd[:, :], smaxAll[:, :])

    nc.compile()
    return nc


_NC_CACHE = {}


def _get_nc():
    if "nc" not in _NC_CACHE:
        _NC_CACHE["nc"] = build_nc()
    return _NC_CACHE["nc"]


def make_in_maps(Q, K, V, omega):
    bf = ml_dtypes.bfloat16
    Q = np.ascontiguousarray(np.asarray(Q, dtype=np.float32))
    K = np.ascontiguousarray(np.asarray(K, dtype=np.float32))
    V = np.ascontiguousarray(np.asarray(V, dtype=np.float32))
    omega = np.asarray(omega, dtype=np.float32)

    QT = Q.T.astype(bf)
    KT = K.T.astype(bf)
    omega_s = (omega * np.float32(D ** -0.25)).astype(bf)
    hscale = np.float32(1.0 / (2.0 * math.sqrt(D)))
    hq = (Q * Q).sum(axis=1) * hscale            # [N]
    hk = (K * K).sum(axis=1) * hscale
    ehk = np.exp(-hk).astype(np.float32)          # [N]
    Vb = V.astype(bf).astype(np.float32)
    VP = (ehk[:, None] * Vb).astype(bf)           # V' rows scaled
    # eps * per-segment colsum of raw [V] (bf16-rounded V)
    nseg_tot = N_CORES * NSEG
    cvs = (EPS_PHI * Vb.reshape(nseg_tot, 512, DV).sum(axis=1)).astype(bf)
    ident = np.eye(P, dtype=np.float32).astype(bf)

    def cols(x):   # [N] -> per-core [P, NSEG*CH] with x[s*512+c*128+p]
        return np.ascontiguousarray(
            x.reshape(N_CORES, NSEG, CH, P).transpose(0, 3, 1, 2)
            .reshape(N_CORES, P, NSEG * CH))

    hqm = cols(-hq).astype(np.float32)
    ehkc = cols(ehk).astype(bf)

    in_maps = []
    for c in range(N_CORES):
        sl = slice(c * TOK, (c + 1) * TOK)
        in_maps.append({
            "QT": np.ascontiguousarray(QT[:, sl]),
            "KT": np.ascontiguousarray(KT[:, sl]),
            "VP": VP[sl],
            "omega": omega_s,
            "HQM": hqm[c],
            "EHK": ehkc[c],
            "CVS": np.ascontiguousarray(
                cvs[c * NSEG:(c + 1) * NSEG].reshape(1, NSEG * DV)),
            "identb": ident,
        })
    return in_maps


def assemble(results):
    outs = []
    for c in range(N_CORES):
        r = results[c]
        num = np.asarray(r["num"], dtype=np.float32)          # [TOK, 256]
        den = r["den"].reshape(P, NSEG, CH).transpose(1, 2, 0).reshape(TOK)
        smax = r["smax"].reshape(NSEG)                        # e^{segmax}
        den = den + (M * EPS_NORM) * np.repeat(smax, 512)
        outs.append(num / den[:, None])
    return np.concatenate(outs, axis=0).astype(np.float32)


def kernel(Q, K, V, omega, num_batch, batch_seg):
    nc = _get_nc()
    in_maps = make_in_maps(Q, K, V, omega)
    res = run_bass_kernel_spmd(nc, in_maps, core_ids=list(range(N_CORES)))
    return assemble(res.results)


# revision 16
# speedup vs baseline: 1.1499x; 1.1499x over previous
"""Trainium2 Bass kernel for segmented linear (performer-style) attention.

Problem: nn_Attention_43550968382196 (sparse_attention).
  N=32768 tokens in 64 contiguous equal segments of 512, d_qk=128, d_v=256,
  m=256 random features.  Per segment:
     phi_q = (exp(Uq - hq - rowmax(Uq)) + eps) / sqrt(m)
     phi_k = (exp(Uk - hk - segmax(Uk)) + eps) / sqrt(m)
     out   = (phi_q @ (phi_k^T V)) / (phi_q . sum(phi_k) + 1e-8)

Device math (v5; validated 4.8e-3 rel err vs the jax reference in numpy):
  * All matmuls bf16 operands, fp32 PSUM accumulation (fp8 was tested and
    fails the 2e-2 gate: e4m3's 6% per-element error survives averaging).
  * Q side: Qp = exp(Uq - hq - mx) via one Act pass per chunk (bias AP);
    the +eps rides the PSUM->SBUF copy after the PE transpose (Copy with
    float bias / tensor_scalar_add), so no eps rank-1 on the Q side.
  * K side: e^{-hk} is folded into V rows ON THE HOST (V' = e^{-hk} V), so
    phi_k-dev = exp(Uk) needs NO bias: one exp covers 2 chunks.  The
    segment max enters only through the eps correction: segmax' =
    max(exp(Uk)) = e^{segmax} via a gpsimd all-dims reduce, used as the
    rank-1 lhsT scale: KV += segmax' * (eps * colsum_raw(V)), Ksum +=
    segmax' * eps*512.  Per-segment scale e^{segmax} cancels in the ratio.
  * den is a separate 1-column matmul chain; num/den are DMA'd out in raw
    fp32 straight from PSUM and the division (+ the 1e-8*m*segmax' norm
    epsilon) happens on the host.

Sharding: 64 segments split 8-per-core across 8 NeuronCores (data parallel,
no collectives); each core runs this program on its 4096-token shard.
"""

import math
import os
import sys

for _p in ("/opt/trn_rl_repo",):
    if _p not in sys.path and os.path.isdir(_p):
        sys.path.insert(0, _p)

import numpy as np
import ml_dtypes

import concourse.bass as bass
import concourse.bacc as bacc
import concourse.tile as tile
from concourse import mybir
from concourse.bass_utils import run_bass_kernel_spmd

F32 = mybir.dt.float32
BF16 = mybir.dt.bfloat16
AF = mybir.ActivationFunctionType
ALU = mybir.AluOpType
AX = mybir.AxisListType

N_CORES = 8
N = 32768
D = 128          # qk dim
M = 256          # features
DV = 256         # v dim
P = 128          # partitions / tokens per chunk
NSEG = 8         # segments per core
CH = 4           # chunks per segment
MC = 2           # m chunks (256 / 128)
TOK = NSEG * 512

EPS_PHI = 1e-4
EPS_NORM = 1e-8


def build_nc():
    nc = bacc.Bacc("TRN2", target_bir_lowering=False, debug=False)

    QTd = nc.declare_dram_parameter("QT", [D, TOK], BF16, isOutput=False)
    KTd = nc.declare_dram_parameter("KT", [D, TOK], BF16, isOutput=False)
    Vd = nc.declare_dram_parameter("VP", [TOK, DV], BF16, isOutput=False)
    Wd = nc.declare_dram_parameter("omega", [D, M], BF16, isOutput=False)
    HQd = nc.declare_dram_parameter("HQM", [P, NSEG * CH], F32, isOutput=False)
    EHd = nc.declare_dram_parameter("EHK", [P, NSEG * CH], BF16, isOutput=False)
    CVd = nc.declare_dram_parameter("CVS", [1, NSEG * DV], BF16, isOutput=False)
    Id = nc.declare_dram_parameter("identb", [P, P], BF16, isOutput=False)
    Od = nc.declare_dram_parameter("num", [TOK, DV], BF16, isOutput=True)
    Dd = nc.declare_dram_parameter("den", [P, NSEG * CH], F32, isOutput=True)
    Sd = nc.declare_dram_parameter("smax", [1, NSEG], F32, isOutput=True)

    Vv = Vd[:, :].rearrange("(s c p) d -> s p c d", s=NSEG, c=CH, p=P)
    Ov = Od[:, :].rearrange("(s c p) d -> s p c d", s=NSEG, c=CH, p=P)

    with tile.TileContext(nc) as tc:
        with (
            tc.tile_pool(name="const", bufs=1) as const,
            tc.tile_pool(name="sb", bufs=2) as sb,
            tc.tile_pool(name="sm", bufs=3) as sm,
            tc.tile_pool(name="ps", bufs=1, space="PSUM") as ps,
        ):
            # PE warm-up: ~5us of dummy matmuls with no input deps so the
            # tensor engine p-state ramps to 2.4GHz during the DMA fill.
            scr1 = const.tile([P, 1], BF16, name="scr1")
            nc.vector.memset(scr1[:, :], 1.0)
            scr2 = const.tile([P, 512], BF16, name="scr2")
            nc.vector.memset(scr2[:, :], 1.0)

            # segment-0 inputs first, then consts, then the rest
            qT_all = const.tile([D, TOK], BF16, name="qT_all")
            kT_all = const.tile([D, TOK], BF16, name="kT_all")
            vp_all = const.tile([P, NSEG, CH, DV], BF16, name="vp_all")
            nc.sync.dma_start(kT_all[:, 0:512], KTd[:, 0:512])
            nc.sync.dma_start(qT_all[:, 0:512], QTd[:, 0:512])
            nc.sync.dma_start(vp_all[:, 0], Vv[0])
            omega_t = const.tile([D, M], BF16, name="omega_t")
            nc.sync.dma_start(omega_t[:, :], Wd[:, :])
            ident_t = const.tile([P, P], BF16, name="ident_t")
            nc.sync.dma_start(ident_t[:, :], Id[:, :])
            hqm_t = const.tile([P, NSEG, CH], F32, name="hqm_t")
            nc.sync.dma_start(
                hqm_t[:, :, :],
                HQd[:, :].rearrange("p (s c) -> p s c", s=NSEG))
            ehk_t = const.tile([P, NSEG, CH], BF16, name="ehk_t")
            nc.sync.dma_start(
                ehk_t[:, :, :],
                EHd[:, :].rearrange("p (s c) -> p s c", s=NSEG))
            cvs_t = const.tile([1, NSEG, DV], BF16, name="cvs_t")
            nc.sync.dma_start(
                cvs_t[:, :, :],
                CVd[:, :].rearrange("p (s d) -> p s d", s=NSEG))
            c512_t = const.tile([1, 1], BF16, name="c512_t")
            nc.vector.memset(c512_t[:, :], EPS_PHI * 512.0)
            # outputs accumulated in SBUF, one DMA each at the end
            denAll = const.tile([P, NSEG, CH], F32, name="denAll")
            smaxAll = const.tile([1, NSEG], F32, name="smaxAll")

            # remaining per-segment input loads
            for s in range(1, NSEG):
                sl = bass.ts(s, 512)
                nc.sync.dma_start(kT_all[:, sl], KTd[:, sl])
                nc.sync.dma_start(qT_all[:, sl], QTd[:, sl])
                nc.sync.dma_start(vp_all[:, s], Vv[s])

            # warm-up matmuls (output unread; rides the NN psum ring)
            warm = ps.tile([P, 2, DV], F32, name="warm", tag="NN", bufs=2)
            for i in range(24):
                nc.tensor.matmul(warm[0:1, i % 2, :], scr1[:, 0:1],
                                 scr2[:, 0:DV], skip_group_check=True)

            # per-segment state carried between pipeline stages
            st = [None] * NSEG

            def stage1(s):
                # ---- K side first: its smax chain is the longest ------
                uk0 = ps.tile([P, 2, M], F32, name=f"uk0_{s}", tag="U", bufs=3)
                uk1 = ps.tile([P, 2, M], F32, name=f"uk1_{s}", tag="U", bufs=3)
                for c in range(CH):
                    u = (uk0, uk1)[c // 2]
                    nc.tensor.matmul(u[:, c % 2, :],
                                     kT_all[:, bass.ts(s * CH + c, P)],
                                     omega_t[:, :])
                # K: exp with no bias (one op per psU tile)
                kp = sb.tile([P, CH, M], BF16, name=f"kp{s}", tag="kp", bufs=3)
                nc.scalar.activation(kp[:, 0:2, :], uk0[:, :, :], AF.Exp)
                nc.scalar.activation(kp[:, 2:4, :], uk1[:, :, :], AF.Exp)
                # segmax' = max(exp(Uk)) via gpsimd all-reduce (SBUF in)
                smx = sm.tile([1, 1], F32, name=f"smx{s}", tag="smx")
                nc.gpsimd.tensor_reduce(smx[:, :], kp[:, :, :],
                                        axis=AX.XYZWC, op=ALU.max)
                smrow = sm.tile([1, P], BF16, name=f"smrow{s}", tag="smrow")
                nc.vector.tensor_copy(smrow[:, :],
                                      smx[:, :].broadcast_to([1, P]))
                nc.gpsimd.tensor_copy(smaxAll[0:1, s:s + 1], smx[:, :])

                # ---- Q side: U matmuls, rowmax -> bias -> exp ---------
                uq0 = ps.tile([P, 2, M], F32, name=f"uq0_{s}", tag="U", bufs=3)
                uq1 = ps.tile([P, 2, M], F32, name=f"uq1_{s}", tag="U", bufs=3)
                for c in range(CH):
                    u = (uq0, uq1)[c // 2]
                    nc.tensor.matmul(u[:, c % 2, :],
                                     qT_all[:, bass.ts(s * CH + c, P)],
                                     omega_t[:, :])
                mx4 = sm.tile([P, CH], F32, name=f"mx4_{s}", tag="mx4")
                nc.vector.tensor_reduce(mx4[:, 0:2], uq0[:, :, :],
                                        axis=AX.X, op=ALU.max)
                nc.vector.tensor_reduce(mx4[:, 2:4], uq1[:, :, :],
                                        axis=AX.X, op=ALU.max)
                biasq = sm.tile([P, CH], F32, name=f"biasq_{s}", tag="biasq")
                nc.gpsimd.tensor_tensor(biasq[:, :], hqm_t[:, s], mx4[:, :],
                                        op=ALU.subtract)
                qp = sb.tile([P, CH, M], BF16, name=f"qp{s}", tag="qp", bufs=3)
                for c in range(CH):
                    nc.scalar.activation(qp[:, c, :],
                                         (uq0, uq1)[c // 2][:, c % 2, :],
                                         AF.Exp, bias=biasq[:, c:c + 1])
                st[s] = (qp, kp, smrow)

            def stage2a(s):
                qp, kp, smrow = st[s]
                # ---- QpT = T(qp) + eps  (PE transpose, copy adds eps) -
                psT0 = ps.tile([P, 512], BF16, name=f"psT0_{s}", tag="T",
                               bufs=2)
                psT1 = ps.tile([P, 512], BF16, name=f"psT1_{s}", tag="T",
                               bufs=2)
                for c in range(CH):
                    nc.tensor.transpose(psT0[:, bass.ts(c, P)],
                                        qp[:, c, 0:P], ident_t[:, :])
                    nc.tensor.transpose(psT1[:, bass.ts(c, P)],
                                        qp[:, c, P:M], ident_t[:, :])
                qpT = sb.tile([P, MC, 512], BF16, name=f"qpT{s}", tag="qpT",
                              bufs=2)
                nc.scalar.activation(qpT[:, 0, :], psT0[:, :], AF.Copy,
                                     bias=EPS_PHI)
                nc.vector.tensor_scalar_add(qpT[:, 1, :], psT1[:, :],
                                            EPS_PHI)

                # ---- KV = Kp^T V' (+ rank-1 eps) ; Ksum likewise ------
                psKV = ps.tile([P, MC, DV], F32, name=f"psKV{s}", tag="W",
                               bufs=1)
                psKs = ps.tile([P, MC, 1], F32, name=f"psKs{s}", tag="T",
                               bufs=2)
                for mc in range(MC):
                    nc.tensor.matmul(psKV[:, mc, :], smrow[0:1, :],
                                     cvs_t[0:1, s, :], start=True, stop=False)
                    for c in range(CH):
                        nc.tensor.matmul(psKV[:, mc, :],
                                         kp[:, c, bass.ts(mc, P)],
                                         vp_all[:, s, c, :],
                                         start=False, stop=(c == CH - 1))
                    nc.tensor.matmul(psKs[:, mc, :], smrow[0:1, :],
                                     c512_t[0:1, :], start=True, stop=False)
                    for c in range(CH):
                        nc.tensor.matmul(psKs[:, mc, :],
                                         kp[:, c, bass.ts(mc, P)],
                                         ehk_t[:, s, c:c + 1],
                                         start=False, stop=(c == CH - 1))
                kvb = sb.tile([P, MC, DV + 1], BF16, name=f"kvb{s}",
                              tag="kvb", bufs=2)
                nc.vector.tensor_copy(kvb[:, 0, 0:DV], psKV[:, 0, :])
                nc.vector.tensor_copy(kvb[:, 1, 0:DV], psKV[:, 1, :])
                nc.vector.tensor_copy(kvb[:, :, DV:DV + 1], psKs[:, :, :])
                st[s] = (qpT, kvb)

            def stage2b(s):
                qpT, kvb = st[s]
                # ---- num / den matmuls + stores -----------------------
                psD = ps.tile([P, CH], F32, name=f"psD{s}", tag="T", bufs=2)
                for half in range(2):
                    psN = ps.tile([P, 2, DV], F32, name=f"psN{s}_{half}",
                                  tag="NN", bufs=2)
                    for i in range(2):
                        c = half * 2 + i
                        for mc in range(MC):
                            nc.tensor.matmul(psN[:, i, :],
                                             qpT[:, mc, bass.ts(c, P)],
                                             kvb[:, mc, 0:DV],
                                             start=(mc == 0), stop=(mc == 1))
                        for mc in range(MC):
                            nc.tensor.matmul(psD[:, c:c + 1],
                                             qpT[:, mc, bass.ts(c, P)],
                                             kvb[:, mc, DV:DV + 1],
                                             start=(mc == 0), stop=(mc == 1))
                    numb = sb.tile([P, 2, DV], BF16, name=f"numb{s}_{half}",
                                   tag="numb", bufs=3)
                    if half == 0:
                        nc.scalar.activation(numb[:, :, :], psN[:, :, :],
                                             AF.Copy)
                    else:
                        nc.vector.tensor_copy(numb[:, :, :], psN[:, :, :])
                    nc.sync.dma_start(Ov[s, :, 2 * half:2 * half + 2, :],
                                      numb[:, :, :])
                nc.vector.tensor_copy(denAll[:, s, :], psD[:, :])

            for s in range(NSEG):
                if s > 0:
                    stage2a(s - 1)
                stage1(s)
                if s > 0:
                    stage2b(s - 1)
            stage2a(NSEG - 1)
            stage2b(NSEG - 1)

            nc.sync.dma_start(Dd[:, :],
                              denAll[:, :, :].rearrange("p s c -> p (s c)"))
            nc.sync.dma_start(Sd[:, :], smaxAll[:, :])

    nc.compile()
    return nc


_NC_CACHE = {}


def _get_nc():
    if "nc" not in _NC_CACHE:
        _NC_CACHE["nc"] = build_nc()
    return _NC_CACHE["nc"]


def make_in_maps(Q, K, V, omega):
    bf = ml_dtypes.bfloat16
    Q = np.ascontiguousarray(np.asarray(Q, dtype=np.float32))
    K = np.ascontiguousarray(np.asarray(K, dtype=np.float32))
    V = np.ascontiguousarray(np.asarray(V, dtype=np.float32))
    omega = np.asarray(omega, dtype=np.float32)

    QT = Q.T.astype(bf)
    KT = K.T.astype(bf)
    omega_s = (omega * np.float32(D ** -0.25)).astype(bf)
    hscale = np.float32(1.0 / (2.0 * math.sqrt(D)))
    hq = (Q * Q).sum(axis=1) * hscale            # [N]
    hk = (K * K).sum(axis=1) * hscale
    ehk = np.exp(-hk).astype(np.float32)          # [N]
    Vb = V.astype(bf).astype(np.float32)
    VP = (ehk[:, None] * Vb).astype(bf)           # V' rows scaled
    # eps * per-segment colsum of raw [V] (bf16-rounded V)
    nseg_tot = N_CORES * NSEG
    cvs = (EPS_PHI * Vb.reshape(nseg_tot, 512, DV).sum(axis=1)).astype(bf)
    ident = np.eye(P, dtype=np.float32).astype(bf)

    def cols(x):   # [N] -> per-core [P, NSEG*CH] with x[s*512+c*128+p]
        return np.ascontiguousarray(
            x.reshape(N_CORES, NSEG, CH, P).transpose(0, 3, 1, 2)
            .reshape(N_CORES, P, NSEG * CH))

    hqm = cols(-hq).astype(np.float32)
    ehkc = cols(ehk).astype(bf)

    in_maps = []
    for c in range(N_CORES):
        sl = slice(c * TOK, (c + 1) * TOK)
        in_maps.append({
            "QT": np.ascontiguousarray(QT[:, sl]),
            "KT": np.ascontiguousarray(KT[:, sl]),
            "VP": VP[sl],
            "omega": omega_s,
            "HQM": hqm[c],
            "EHK": ehkc[c],
            "CVS": np.ascontiguousarray(
                cvs[c * NSEG:(c + 1) * NSEG].reshape(1, NSEG * DV)),
            "identb": ident,
        })
    return in_maps


def assemble(results):
    outs = []
    for c in range(N_CORES):
        r = results[c]
        num = np.asarray(r["num"], dtype=np.float32)          # [TOK, 256]
        den = r["den"].reshape(P, NSEG, CH).transpose(1, 2, 0).reshape(TOK)
        smax = r["smax"].reshape(NSEG)                        # e^{segmax}
        den = den + (M * EPS_NORM) * np.repeat(smax, 512)
        outs.append(num / den[:, None])
    return np.concatenate(outs, axis=0).astype(np.float32)


def kernel(Q, K, V, omega, num_batch, batch_seg):
    nc = _get_nc()
    in_maps = make_in_maps(Q, K, V, omega)
    res = run_bass_kernel_spmd(nc, in_maps, core_ids=list(range(N_CORES)))
    return assemble(res.results)
